# revision 19
# baseline (speedup 1.0000x reference)
"""GQA attention block (QKV proj + causal attention + output proj) on 8 trn2 cores.

Sharding: core c -> (batch b = c//4, kv-group g = c%4). Each core computes 4 Q
heads (one KV-head group) of one batch and a partial o_proj output; the host
sums the 4 partials per batch (row-sharded o_proj all-reduce done host-side).

All matmul operands are bf16 (PE runs 4x faster than fp32; PSUM accumulation
stays fp32). Attention uses transposed scores S^T[tk, tq] so the softmax
denominator comes for free from a ones-column appended to V.

The kernel is software-pipelined around the Activation engine: exp of the
score tiles is the per-tile bottleneck in the attention phase (Act ~1.25ns/col
vs PE ~0.417ns/row), and every engine queue is in-order. So PE filler work
(Q-projection chains for the next tq chunk, o_proj of the previous chunk) is
emitted *between* each score matmul and its attn@V matmuls, keeping the PE
busy while Act exponentiates. Input DMA is issued ko-major so the lead-in
projection chains start as soon as each contraction subtile lands, and the
y^T layout fix-up runs on the (otherwise idle) DMA transpose path instead of
the PE.
"""

import math
from collections import deque

import numpy as np

# Model dims (hardcoded per contract; kernel.py must be self-contained).
B = 2
T = 2048
E = 2048
HD = 128               # head dim
NH = 16                # query heads total
NKV = 4                # kv heads total
NHC = 4                # query heads per core
P = 128
KO = E // P            # 16 contraction subtiles of 128
TQC = T // 512         # 4 query chunks of 512
TB = T // P            # 16 t blocks of 128
SCALE = 1.0 / math.sqrt(HD)
N_CORES = 8

_NC_CACHE = {}


def _build_nc():
    import concourse.bacc as bacc
    import concourse.mybir as mybir
    import concourse.tile as tile
    from concourse.masks import make_upper_triangular

    f32 = mybir.dt.float32
    bf16 = mybir.dt.bfloat16
    nc = bacc.Bacc(None, target_bir_lowering=False)

    xT = nc.dram_tensor("xT", [E, T], bf16, kind="ExternalInput")
    wqT = nc.dram_tensor("wqT", [E, NHC * HD], bf16, kind="ExternalInput")
    wkT = nc.dram_tensor("wkT", [E, HD], bf16, kind="ExternalInput")
    wvT = nc.dram_tensor("wvT", [E, HD], bf16, kind="ExternalInput")
    woT = nc.dram_tensor("woT", [NHC * HD, E], bf16, kind="ExternalInput")
    out = nc.dram_tensor("out", [T, E], f32, kind="ExternalOutput")

    xT_r = xT.rearrange("(ko p) t -> p ko t", p=P)        # [128, 16, T]
    wqT_r = wqT.rearrange("(ko p) d -> p ko d", p=P)      # [128, 16, 512]
    wkT_r = wkT.rearrange("(ko p) d -> p ko d", p=P)      # [128, 16, 128]
    wvT_r = wvT.rearrange("(ko p) d -> p ko d", p=P)
    woT_r = woT.rearrange("(h p) e -> p h e", p=P)        # [128, 4, E]
    out_r = out.rearrange("(tb p) e -> p tb e", p=P)      # [128, 16, E]

    with tile.TileContext(nc) as tc:
        with (
            tc.tile_pool(name="const", bufs=1) as constp,
            tc.tile_pool(name="big", bufs=1) as bigp,
            tc.tile_pool(name="work", bufs=6) as work,
            tc.tile_pool(name="owork", bufs=2) as owork,
            # PSUM: 2 + 4 + 2 = 8 banks.
            tc.tile_pool(name="ps_s", bufs=2, space="PSUM") as ps_s,
            tc.tile_pool(name="ps_c", bufs=2, space="PSUM") as ps_c,
            tc.tile_pool(name="ps_y", bufs=4, space="PSUM") as ps_y,
        ):
            # tri[p, q] = 1.0 where p <= q — causal mask for the one
            # tk==tq diagonal 128x128 sub-block.
            tri = constp.tile([P, P], bf16, tag="tri")
            make_upper_triangular(nc, tri[:], val=1.0, diag=True)

            XT = bigp.tile([P, KO, T], bf16, tag="XT")
            WQT = bigp.tile([P, KO, NHC * HD], bf16, tag="WQT")
            WKT = bigp.tile([P, KO, HD], bf16, tag="WKT")
            WVT = bigp.tile([P, KO, HD], bf16, tag="WVT")
            WOT = bigp.tile([P, NHC, E], bf16, tag="WOT")
            QT = bigp.tile([P, NHC, T], bf16, tag="QT")    # q^T per head [d, t]
            KT = bigp.tile([P, T], bf16, tag="KT")         # k^T [d, t]
            VAUG = bigp.tile([P, TB, HD + 1], bf16, tag="VAUG")  # v blocks [tk,129]
            YT = bigp.tile([P, NHC, T], bf16, tag="YT")    # y^T per head [d, t]

            nc.vector.memset(VAUG[:, :, HD:HD + 1], 1.0)

            # DMA issue order is the pacing order. Few, large transfers:
            # HWDGE costs ~625ns per DMA *instruction*, so weights load in
            # single batched DMAs; only x streams per-ko to pace the lead-in
            # chains. Q heads 0-1 slices interleave every 4 ko.
            nc.sync.dma_start(WKT[:], wkT_r[:])
            for g in range(4):
                for ko in range(4 * g, 4 * g + 4):
                    nc.sync.dma_start(XT[:, ko], xT_r[:, ko])
                nc.sync.dma_start(
                    WQT[:, 4 * g:4 * g + 4, 0:2 * HD],
                    wqT_r[:, 4 * g:4 * g + 4, 0:2 * HD],
                )
            nc.sync.dma_start(WVT[:], wvT_r[:])
            nc.sync.dma_start(WQT[:, :, 2 * HD:4 * HD], wqT_r[:, :, 2 * HD:4 * HD])
            nc.sync.dma_start(WOT[:], woT_r[:])

            # ---- Projection chain emitters: each chain is a list of
            # (pe_rows, emit_fn) single-matmul units; the last unit also
            # emits the PSUM->SBUF drain.
            def q_chain_units(h, tcol, pool, tag, drain_act=False):
                ps = pool.tile([P, 512], f32, tag=tag)

                def unit(ko, ps=ps, h=h, tcol=tcol):
                    nc.tensor.matmul(
                        ps[:],
                        WQT[:, ko, h * HD:(h + 1) * HD],
                        XT[:, ko, tcol * 512:(tcol + 1) * 512],
                        start=(ko == 0),
                        stop=(ko == KO - 1),
                    )
                    if ko == KO - 1:
                        # GPSIMD can't read PSUM; drain on Act during the
                        # lead-in (it's idle there), DVE during attention.
                        dst = QT[:, h, tcol * 512:(tcol + 1) * 512]
                        if drain_act:
                            nc.scalar.copy(dst, ps[:])
                        else:
                            nc.vector.tensor_copy(dst, ps[:])
                return [(512, lambda ko=ko: unit(ko)) for ko in range(KO)]

            def k_chain_units(tcol, drain_act=False):
                ps = ps_c.tile([P, 512], f32, tag="ps_c")

                def unit(ko, ps=ps, tcol=tcol):
                    nc.tensor.matmul(
                        ps[:],
                        WKT[:, ko],
                        XT[:, ko, tcol * 512:(tcol + 1) * 512],
                        start=(ko == 0),
                        stop=(ko == KO - 1),
                    )
                    if ko == KO - 1:
                        dst = KT[:, tcol * 512:(tcol + 1) * 512]
                        if drain_act:
                            nc.scalar.copy(dst, ps[:])
                        else:
                            nc.vector.tensor_copy(dst, ps[:])
                return [(512, lambda ko=ko: unit(ko)) for ko in range(KO)]

            def v_chain_units(tb, ps, drain_act=False):
                # v-block (natural [t, d] layout) accumulation chain. Only
                # ONE live accumulation chain per PSUM bank: hardware
                # corrupts a bank region if another chain starts in the
                # same bank mid-flight, so sequential chains reuse the SAME
                # region (cols 0:128) — the framework's WAR dependency on
                # the drain read serializes them safely.
                def unit(ko, tb=tb, ps=ps):
                    nc.tensor.matmul(
                        ps[:, 0:P],
                        XT[:, ko, tb * P:(tb + 1) * P],
                        WVT[:, ko],
                        start=(ko == 0),
                        stop=(ko == KO - 1),
                    )
                    if ko == KO - 1:
                        if drain_act:
                            nc.scalar.copy(VAUG[:, tb, 0:HD], ps[:, 0:P])
                        else:
                            nc.vector.tensor_copy(VAUG[:, tb, 0:HD], ps[:, 0:P])
                return [(128, lambda ko=ko: unit(ko)) for ko in range(KO)]

            def oproj_chain(tb, ec, osb4):
                # whole 4-matmul chain as one filler item: short PSUM slot
                # hold; the tb's 4 chains share one [128, 2048] staging tile
                # flushed by a single output DMA.
                ps = ps_c.tile([P, 512], f32, tag="ps_c")

                def emit(ps=ps, tb=tb, ec=ec, osb4=osb4):
                    for h in range(NHC):
                        nc.tensor.matmul(
                            ps[:],
                            YT[:, h, tb * P:(tb + 1) * P],
                            WOT[:, h, ec * 512:(ec + 1) * 512],
                            start=(h == 0),
                            stop=(h == NHC - 1),
                        )
                    nc.vector.tensor_copy(osb4[:, ec * 512:(ec + 1) * 512], ps[:])
                    if ec == NHC - 1:
                        nc.sync.dma_start(out_r[:, tb], osb4[:])
                return [(NHC * 512, emit)]

            # ---- Lead-in (paced by the x DMA). 8 concurrent chains — one
            # per PSUM bank: K0,K1 (ps_c), Q00,Q10 (ps_s), V0-V3 (ps_y).
            vtiles = [ps_y.tile([P, HD + 1], f32, tag="ps_y",
                                name=f"vps_{j}") for j in range(4)]
            wave1 = [k_chain_units(0, True), k_chain_units(1, True),
                     q_chain_units(0, 0, ps_s, "ps_s", True),
                     q_chain_units(1, 0, ps_s, "ps_s", True)]
            wave1 += [v_chain_units(j, vtiles[j], True) for j in range(4)]
            for ko in range(KO):
                for ch in wave1:
                    ch[ko][1]()
            # wave 2: K2,K3 + Q heads 2-3 of tcol 0 + V4-V7 (x resident now).
            wave2 = [k_chain_units(2, True), k_chain_units(3, True),
                     q_chain_units(2, 0, ps_s, "ps_s", True),
                     q_chain_units(3, 0, ps_s, "ps_s", True)]
            vtiles2 = [ps_y.tile([P, HD + 1], f32, tag="ps_y",
                                 name=f"vps2_{j}") for j in range(4)]
            wave2 += [v_chain_units(4 + j, vtiles2[j], True) for j in range(4)]
            for ko in range(KO):
                for ch in wave2:
                    ch[ko][1]()
            # remaining v blocks: sequential chains reusing two ps_c banks.
            for tb in range(8, TB):
                psv = ps_c.tile([P, 512], f32, tag="ps_c")
                for _, emit in v_chain_units(tb, psv, True):
                    emit()

            # ---- Filler queue: PE work to hide Act(exp) latency with.
            fillers = deque()

            def drain_fillers(n_rows):
                while n_rows > 0 and fillers:
                    rows, emit = fillers.popleft()
                    emit()
                    n_rows -= rows

            def drain_all_fillers():
                while fillers:
                    fillers.popleft()[1]()

            # ---- Attention + interleaved projections/o_proj.
            for tqc in range(TQC):
                if tqc + 1 < TQC:
                    for h in range(NHC):
                        fillers.extend(q_chain_units(h, tqc + 1, ps_c, "ps_c"))
                for h in range(NHC):
                    ntk = 4 * (tqc + 1)   # tk blocks up to the diagonal
                    psy = [
                        ps_y.tile([P, HD + 1], f32, tag="ps_y",
                                  name=f"psy_{j}")
                        for j in range(4)
                    ]
                    # scores+exp for tile tk are emitted one iteration ahead
                    # of the exp-dependent attn@V matmuls of tile tk-1, so
                    # the PE computes scores (and fillers) while Act
                    # exponentiates — the engines pipeline instead of
                    # serial-chaining.
                    es_q = deque()

                    def emit_scores(tk):
                        i = tk - 4 * tqc
                        off = max(0, i) * P
                        w = 512 - off
                        pss = ps_s.tile([P, 512], f32, tag="ps_s")
                        nc.tensor.matmul(
                            pss[:, 0:w],
                            KT[:, tk * P:(tk + 1) * P],
                            QT[:, h, tqc * 512 + off:(tqc + 1) * 512],
                            start=True,
                            stop=True,
                        )
                        es = work.tile([P, 512], bf16, tag="expS")
                        nc.scalar.activation(
                            es[:, 0:w], pss[:, 0:w],
                            mybir.ActivationFunctionType.Exp,
                            scale=SCALE,
                        )
                        if i >= 0:
                            nc.vector.tensor_mul(
                                out=es[:, 0:P], in0=es[:, 0:P], in1=tri[:]
                            )
                        es_q.append(es)

                    emit_scores(0)
                    for tk in range(ntk):
                        i = tk - 4 * tqc  # >= 0 inside the diagonal region
                        off = max(0, i) * P   # local tq offset of valid cols
                        w = 512 - off
                        if tk + 1 < ntk:
                            emit_scores(tk + 1)
                        # fill the Act-PE gap: exp costs ~3 PE-rows per col,
                        # the tile's own PE work is w + (4-j0)*129 rows.
                        drain_fillers(2 * w + 200 - (4 - max(0, i)) * 129)
                        es = es_q.popleft()
                        for j in range(max(0, i), 4):
                            nc.tensor.matmul(
                                psy[j][:],
                                es[:, j * P - off:(j + 1) * P - off],
                                VAUG[:, tk],
                                start=(tk == 0),
                                stop=(tk == 4 * tqc + j),
                            )
                    for j in range(4):
                        jg = 4 * tqc + j
                        recip = work.tile([P, 1], f32, tag="recip")
                        nc.vector.reciprocal(recip[:], psy[j][:, HD:HD + 1])
                        ysb = work.tile([P, P], bf16, tag="ysb")
                        nc.vector.tensor_scalar_mul(ysb[:], psy[j][:, 0:HD], recip[:])
                        nc.sync.dma_start_transpose(
                            YT[:, h, jg * P:(jg + 1) * P], ysb[:]
                        )
                # Q chains for the next chunk must be fully emitted before
                # that chunk's scores read QT.
                drain_all_fillers()
                # o_proj for the finished chunk becomes next round's filler.
                for tb in range(4 * tqc, 4 * tqc + 4):
                    osb4 = owork.tile([P, E], f32, tag="osb4")
                    for ec in range(4):
                        fillers.extend(oproj_chain(tb, ec, osb4))
            drain_all_fillers()

    nc.finalize()
    return nc


def _get_nc():
    if "nc" not in _NC_CACHE:
        _NC_CACHE["nc"] = _build_nc()
    return _NC_CACHE["nc"]


def _in_maps(x, wq, wk, wv, wo):
    import ml_dtypes

    bf16 = ml_dtypes.bfloat16
    xTb = [np.ascontiguousarray(x[b].astype(bf16).T) for b in range(B)]
    wqT = [
        np.ascontiguousarray(wq[g * 512:(g + 1) * 512].astype(bf16).T)
        for g in range(NKV)
    ]
    wkT = [
        np.ascontiguousarray(wk[g * HD:(g + 1) * HD].astype(bf16).T)
        for g in range(NKV)
    ]
    wvT = [
        np.ascontiguousarray(wv[g * HD:(g + 1) * HD].astype(bf16).T)
        for g in range(NKV)
    ]
    woT = [
        np.ascontiguousarray(wo[:, g * 512:(g + 1) * 512].astype(bf16).T)
        for g in range(NKV)
    ]
    maps = []
    for c in range(N_CORES):
        b, g = divmod(c, NKV)
        maps.append({
            "xT": xTb[b],
            "wqT": wqT[g],
            "wkT": wkT[g],
            "wvT": wvT[g],
            "woT": woT[g],
        })
    return maps


def kernel(x, wq, wk, wv, wo):
    from concourse.bass_utils import run_bass_kernel_spmd

    x = np.asarray(x, dtype=np.float32)
    wq = np.asarray(wq, dtype=np.float32)
    wk = np.asarray(wk, dtype=np.float32)
    wv = np.asarray(wv, dtype=np.float32)
    wo = np.asarray(wo, dtype=np.float32)

    nc = _get_nc()
    in_maps = _in_maps(x, wq, wk, wv, wo)

    res = run_bass_kernel_spmd(nc, in_maps, core_ids=list(range(N_CORES)))

    partials = [res.results[c]["out"] for c in range(N_CORES)]
    out = np.empty((B, T, E), dtype=np.float32)
    for b in range(B):
        acc = partials[NKV * b].astype(np.float32)
        for g in range(1, NKV):
            acc = acc + partials[NKV * b + g]
        out[b] = acc
    return out
